# revision 25
# baseline (speedup 1.0000x reference)
"""Trainium2 Bass kernel for GPT2-style single attention layer.

Problem: B=4, S=2048, E=1024, H=16 heads, D=64.
  x = hidden @ W_attn + b_attn ; q,k,v = split(x)
  per head: softmax(causal(q k^T / 8) + mask) @ v
  out = merge @ W_proj + b_proj

Sharding over 8 cores: core i -> batch b = i//2, heads h0 = (i%2)*8 .. +8
(data parallel on B, tensor parallel over heads).  Each core's work is fully
local; the host sums the two partial projections per batch.

Dataflow is fully "transposed" so no on-chip transposes are ever needed:
  host feeds xT = hidden[b].T                       [E, S]
  Q^T,K^T = (Wq|Wk block)^T @ xT     -> [d, tok] per head   (W stationary)
  V       = xT_block^T @ Wv          -> [tok, d] natural    (xT stationary)
  S^T     = K^T_blk^T @ Q^T          -> [k, q]   (softmax dim on partitions)
  P^T     = exp(0.125*S^T + mask[k]) * causal01
  sums    = ones^T @ P^T             -> [1, q]  (ones-matmul, PSUM-accum)
  attn^T  = V_blk^T @ P^T            -> [d, q]  accumulated over k tiles
  norm    = attn^T * broadcast(1/sums)   (broadcast via K=1 ones-matmul)
  out^T   = Wp_blk^T @ attn^T        -> [col, tok]
Host transposes out^T back and sums core pairs + b_proj.

All matmuls run as float32r (full-rate fp32 path, 1 cycle/row at N>=256).
"""

import os
import ml_dtypes
import numpy as np

B, S, E, H, D = 4, 2048, 1024, 16, 64
NC = 8
HL = H // 2          # local heads per core
EL = HL * D          # local embedding slice = 512
P = 128              # partitions
QT = 512             # q tile width (f32 moving max)
NQT = S // QT        # 4 q tiles
NKT = S // P         # 16 k tiles
NET = E // P         # 8 e (contraction) tiles

_CACHE = {}
LAST_RESULT = None


def _build(has_bv: bool):
    from contextlib import ExitStack

    import concourse.tile as tile
    from concourse import bacc, mybir

    f32 = mybir.dt.float32
    f32r = mybir.dt.bfloat16  # matmul operand dtype (2-byte: full-rate moving operand)
    EXP = mybir.ActivationFunctionType.Exp

    nc = bacc.Bacc(
        "TRN2",
        target_bir_lowering=False,
        debug=False,
        enable_asserts=False,
        num_devices=NC,
    )

    def inp(name, shape, dt=f32):
        return nc.dram_tensor(name, shape, dt, kind="ExternalInput").ap()

    xt_d = inp("xt", [E, S], f32r)
    wq_d = inp("wq", [E, EL], f32r)
    wk_d = inp("wk", [E, EL], f32r)
    wv_d = inp("wv", [E, EL], f32r)
    wp_d = inp("wp", [EL, E], f32r)
    bq_d = inp("bq", [P, 4])
    bk_d = inp("bk", [P, 4])
    bv_d = inp("bv", [P, 4])
    maskt_d = inp("maskt", [P, NKT])
    causal_d = inp("causal", [P, P], f32r)
    ones_d = inp("ones", [P, 64], f32r)
    out_d = nc.dram_tensor("out", [E, S], f32, kind="ExternalOutput").ap()


    with tile.TileContext(nc) as tc, ExitStack() as ctx:
        const = ctx.enter_context(tc.tile_pool(name="const", bufs=1))
        big = ctx.enter_context(tc.tile_pool(name="big", bufs=1))
        wpool = ctx.enter_context(tc.tile_pool(name="wpool", bufs=1))
        xpool = ctx.enter_context(tc.tile_pool(name="xpool", bufs=1))
        ptpool = ctx.enter_context(tc.tile_pool(name="ptpool", bufs=1))
        aopool = ctx.enter_context(tc.tile_pool(name="aopool", bufs=1))
        ospool = ctx.enter_context(tc.tile_pool(name="ospool", bufs=1))
        rcpool = ctx.enter_context(tc.tile_pool(name="rcpool", bufs=1))
        aospool = ctx.enter_context(tc.tile_pool(name="aospool", bufs=1))
        psum = ctx.enter_context(tc.tile_pool(name="psum", bufs=1, space="PSUM"))


        # ---- persistent big buffers ----
        # Q^T / K^T: per head-pair p a [128, S] tile (partitions = 2 heads x 64 d)
        qt_tiles = [big.tile([P, S], f32r, name=f"qt{p}", tag=f"qt{p}") for p in range(4)]
        kt_tiles = [big.tile([P, S], f32r, name=f"kt{p}", tag=f"kt{p}") for p in range(4)]
        # V natural: 16 tiles [128 tok, 512 vcol]
        v_tiles = [big.tile([P, 8 * 65], f32r, name=f"v{t}", tag=f"v{t}") for t in range(NKT)]
        # W_proj: 4 partition tiles [128 elocal, 1024 col]
        wp_tiles = [big.tile([P, E], f32r, name=f"wp{p}", tag=f"wp{p}") for p in range(4)]

        # ---- weight tiles (shared slots) ----
        def load_w(dram, label):
            tiles = []
            for kt in range(NET):
                w = wpool.tile([P, EL], f32r, name=f"w_{label}{kt}", tag="w", bufs=24)
                nc.sync.dma_start(w[:], dram[kt * P:(kt + 1) * P, :])
                tiles.append(w)
            return tiles

        # DMA emission ordered by first use: wv + xT quarter 0 gate the first
        # compute; the rest stream in behind.
        x_tiles = [[None] * NQT for _ in range(NET)]

        def load_x_quarter(tq):
            for kt in range(NET):
                x = xpool.tile([P, QT], f32r, name=f"x{kt}_{tq}", tag=f"x{kt}_{tq}",
                               bufs=1)
                nc.sync.dma_start(x[:], xt_d[kt * P:(kt + 1) * P, tq * QT:(tq + 1) * QT])
                x_tiles[kt][tq] = x

        wv_t = load_w(wv_d, "v")
        load_x_quarter(0)
        wq_t = load_w(wq_d, "q")
        bq_t = const.tile([P, 4], f32, name="bq_t")
        nc.sync.dma_start(bq_t[:], bq_d[:])
        wk_t = load_w(wk_d, "k")
        bk_t = const.tile([P, 4], f32, name="bk_t")
        nc.sync.dma_start(bk_t[:], bk_d[:])
        maskt_t = const.tile([P, NKT], f32, name="maskt_t")
        nc.sync.dma_start(maskt_t[:], maskt_d[:])
        causal_t = const.tile([P, P], f32r, name="causal_t")
        nc.sync.dma_start(causal_t[:], causal_d[:])
        ones_t = const.tile([P, 64], f32r, name="ones_t")
        nc.sync.dma_start(ones_t[:], ones_d[:])
        bv_t = const.tile([P, 4], f32, name="bv_t")
        nc.sync.dma_start(bv_t[:], bv_d[:])
        for p in range(4):
            nc.sync.dma_start(wp_tiles[p][:], wp_d[p * P:(p + 1) * P, :])
        for tq in range(1, NQT):
            load_x_quarter(tq)

        def qk_pass_pair(tq, xq, ct):
            ps = psum.tile([P, QT], f32, name=f"psq{tq}_{ct}", tag="mm", bufs=2)
            for kt in range(NET):
                nc.tensor.matmul(ps[:], wq_t[kt][:, ct * P:(ct + 1) * P], xq[kt][:],
                                 start=(kt == 0), stop=(kt == NET - 1))
            nc.vector.tensor_scalar_add(
                qt_tiles[ct][:, tq * QT:(tq + 1) * QT], ps[:], bq_t[:, ct:ct + 1])
            ps2 = psum.tile([P, QT], f32, name=f"psk{tq}_{ct}", tag="mm", bufs=2)
            for kt in range(NET):
                nc.tensor.matmul(ps2[:], wk_t[kt][:, ct * P:(ct + 1) * P], xq[kt][:],
                                 start=(kt == 0), stop=(kt == NET - 1))
            nc.vector.tensor_scalar_add(
                kt_tiles[ct][:, tq * QT:(tq + 1) * QT], ps2[:], bk_t[:, ct:ct + 1])

        def v_pass(tq, xq):
            for tt in range(4):
                ps = psum.tile([P, EL], f32, name=f"psv{tq}_{tt}", tag="mm", bufs=2)
                for kt in range(NET):
                    nc.tensor.matmul(
                        ps[:],
                        xq[kt][:, tt * P:(tt + 1) * P],
                        wv_t[kt][:],
                        start=(kt == 0),
                        stop=(kt == NET - 1),
                    )
                vt = v_tiles[tq * 4 + tt]
                v8 = vt[:, 0:520].rearrange("p (a c) -> p a c", a=8, c=65)
                nc.vector.tensor_copy(
                    v8[:, :, 0:64], ps[:].rearrange("p (a c) -> p a c", a=8, c=64)
                )
                nc.sync.dma_start(
                    v8[:, :, 64:65],
                    ones_d[:, 0:8].rearrange("p (a o) -> p a o", a=8, o=1),
                )


        def attention(p, qt, sga, sgb):
            """Head pair p (heads 2p, 2p+1), q tile qt.

            Leaves attnout halves in an SBUF tile (f32) and the softmax
            denominators in rows 32*p of sga/sgb.  Normalization happens
            batched per qt in normalize()."""
            kt_max = 4 * (qt + 1)
            qsl = slice(qt * QT, (qt + 1) * QT)
            # row 64 of each av accumulates the softmax denominator (ones col)
            ava = psum.tile([65, QT], f32, name=f"ava{p}_{qt}", tag="ava", bufs=1)
            avb = psum.tile([65, QT], f32, name=f"avb{p}_{qt}", tag="avb", bufs=1)

            def av_sums(kt, pt, off):
                first, last = kt == 0, kt == kt_max - 1
                vva = v_tiles[kt][:, (2 * p) * 65:(2 * p + 1) * 65]
                vvb = v_tiles[kt][:, (2 * p + 1) * 65:(2 * p + 2) * 65]
                nc.tensor.matmul(ava[:, off:QT], vva, pt[:, off:QT],
                                 start=first, stop=last)
                nc.tensor.matmul(avb[:, off:QT], vvb, pt[:, QT + off:2 * QT],
                                 start=first, stop=last)

            pending = None
            for kt in range(kt_max):
                # diagonal tiles: only q columns >= off are unmasked
                diag = kt >= qt * 4
                off = (kt - qt * 4) * P if diag else 0
                kl = slice(kt * P, (kt + 1) * P)
                qv = slice(qt * QT + off, (qt + 1) * QT)
                st = psum.tile([P, 2 * QT], f32, name=f"st{p}_{qt}_{kt}",
                               tag="st", bufs=2)
                nc.tensor.matmul(st[:, off:QT], kt_tiles[p][0:64, kl],
                                 qt_tiles[p][0:64, qv])
                nc.tensor.matmul(st[:, QT + off:2 * QT], kt_tiles[p][64:128, kl],
                                 qt_tiles[p][64:128, qv])
                pt = ptpool.tile([P, 2 * QT], f32r, name=f"pt{p}_{qt}_{kt}",
                                 tag="pt", bufs=3)
                bias = maskt_t[:, kt:kt + 1]
                if not diag or off == 0:
                    nc.scalar.activation(pt[:], st[:], EXP, bias=bias, scale=0.125)
                else:
                    nc.scalar.activation(pt[:, off:QT], st[:, off:QT], EXP,
                                         bias=bias, scale=0.125)
                    nc.scalar.activation(pt[:, QT + off:2 * QT],
                                         st[:, QT + off:2 * QT], EXP,
                                         bias=bias, scale=0.125)
                if diag:
                    # triangular band at the leading 128 valid columns
                    nc.vector.tensor_mul(pt[:, off:off + P], pt[:, off:off + P],
                                         causal_t[:])
                    nc.vector.tensor_mul(pt[:, QT + off:QT + off + P],
                                         pt[:, QT + off:QT + off + P], causal_t[:])
                if pending is not None:
                    av_sums(*pending)
                pending = (kt, pt, off)
            av_sums(*pending)

            # drain PSUM immediately so the next pair's AV can start
            aos = aospool.tile([P, QT], f32, name=f"aos{p}_{qt}",
                               tag=f"aos{p}", bufs=2)
            nc.vector.tensor_copy(aos[0:64, :], ava[0:64, :])
            nc.vector.tensor_copy(aos[64:128, :], avb[0:64, :])
            row = 32 * p
            nc.vector.tensor_copy(sga[row:row + 1, :], ava[64:65, :])
            nc.vector.tensor_copy(sgb[row:row + 1, :], avb[64:65, :])
            return aos

        def normalize_pair(p, qt, sga, sgb, aos):
            """Immediate normalization of one pair (used for the last quarter)."""
            row = 32 * p
            rcf = rcpool.tile([97, QT], f32, name=f"rcfp{p}_{qt}", tag="rcf", bufs=1)
            rcg = rcpool.tile([97, QT], f32, name=f"rcgp{p}_{qt}", tag="rcg", bufs=1)
            nc.vector.reciprocal_approx_fast(rcf[:], sga[:])
            nc.vector.reciprocal_approx_fast(rcg[:], sgb[:])
            rca = rcpool.tile([97, QT], f32r, name=f"rcap{p}_{qt}", tag="rca", bufs=1)
            rcb = rcpool.tile([97, QT], f32r, name=f"rcbp{p}_{qt}", tag="rcb", bufs=1)
            nc.vector.tensor_copy(rca[:], rcf[:])
            nc.vector.tensor_copy(rcb[:], rcg[:])
            ao = aopool.tile([P, QT], f32r, name=f"aop{p}_{qt}", tag=f"ao{p}", bufs=2)
            for half, rcx in ((0, rca), (1, rcb)):
                rb = psum.tile([64, QT], f32, name=f"rbp{p}_{qt}_{half}",
                               tag="mm", bufs=2)
                nc.tensor.matmul(rb[:], ones_t[row:row + 1, 0:64],
                                 rcx[row:row + 1, :], tile_position=(row, 0))
                nc.vector.tensor_mul(ao[64 * half:64 * (half + 1), :], rb[:],
                                     aos[64 * half:64 * (half + 1), :])
            if has_bv:
                nc.vector.tensor_scalar_add(ao[:], ao[:], bv_t[:, p:p + 1])
            return ao

        def normalize(qt, sga, sgb, aos_tiles):
            """Batched softmax normalization for all 4 pairs of one q tile."""
            rcf = rcpool.tile([97, QT], f32, name=f"rcf{qt}", tag="rcf", bufs=1)
            rcg = rcpool.tile([97, QT], f32, name=f"rcg{qt}", tag="rcg", bufs=1)
            nc.vector.reciprocal_approx_fast(rcf[:], sga[:])
            nc.vector.reciprocal_approx_fast(rcg[:], sgb[:])
            rca = rcpool.tile([97, QT], f32r, name=f"rca{qt}", tag="rca", bufs=1)
            rcb = rcpool.tile([97, QT], f32r, name=f"rcb{qt}", tag="rcb", bufs=1)
            nc.vector.tensor_copy(rca[:], rcf[:])
            nc.vector.tensor_copy(rcb[:], rcg[:])
            ao_tiles = []
            for p in range(4):
                row = 32 * p
                ao = aopool.tile([P, QT], f32r, name=f"ao{p}_{qt}",
                                 tag=f"ao{p}", bufs=2)
                for half, rcx in ((0, rca), (1, rcb)):
                    rb = psum.tile([64, QT], f32, name=f"rb{p}_{qt}_{half}",
                                   tag="mm", bufs=2)
                    nc.tensor.matmul(rb[:], ones_t[row:row + 1, 0:64],
                                     rcx[row:row + 1, :], tile_position=(row, 0))
                    nc.vector.tensor_mul(ao[64 * half:64 * (half + 1), :], rb[:],
                                         aos_tiles[p][64 * half:64 * (half + 1), :])
                if has_bv:
                    nc.vector.tensor_scalar_add(ao[:], ao[:], bv_t[:, p:p + 1])
                ao_tiles.append(ao)
            return ao_tiles

        def proj(qt, ao_tiles):
            for ct in range(NET):
                ps = psum.tile([P, QT], f32, name=f"psp{qt}_{ct}", tag="mm", bufs=2)
                for p in range(4):
                    nc.tensor.matmul(
                        ps[:],
                        wp_tiles[p][:, ct * P:(ct + 1) * P],
                        ao_tiles[p][:],
                        start=(p == 0),
                        stop=(p == 3),
                    )
                osb = ospool.tile([P, QT], f32, name=f"os{qt}_{ct}", tag="os", bufs=2)
                nc.vector.tensor_copy(osb[:], ps[:])
                nc.sync.dma_start(out_d[ct * P:(ct + 1) * P, qt * QT:(qt + 1) * QT],
                                  osb[:])

        # ============ V/K passes interleaved with attention per quarter ============
        # normalize+proj of quarter tq are deferred into tq+1's attention so the
        # DVE/recip chain overlaps PE matmul work.
        pending_np = None
        for tq in range(NQT):
            xq = [x_tiles[kt][tq] for kt in range(NET)]
            v_pass(tq, xq)
            for ct in range(4):
                qk_pass_pair(tq, xq, ct)
            sga = rcpool.tile([97, QT], f32, name=f"sga{tq}", tag="sga", bufs=2)
            sgb = rcpool.tile([97, QT], f32, name=f"sgb{tq}", tag="sgb", bufs=2)
            aos_tiles = []
            for p in range(4):
                aos_tiles.append(attention(p, tq, sga, sgb))
                if p == 1 and pending_np is not None:
                    proj(pending_np[0], normalize(*pending_np))
                    pending_np = None
            if tq < NQT - 1:
                pending_np = (tq, sga, sgb, aos_tiles)
            else:
                ao_last = [normalize_pair(p, tq, sga, sgb, aos_tiles[p])
                           for p in range(4)]
        proj(NQT - 1, ao_last)

    nc.compile()
    return nc


def _causal_tiles():
    """[128, 128] lower-triangular 0/1 band mask (dq >= dk)."""
    dk = np.arange(P)[:, None]
    dq = np.arange(P)[None, :]
    return np.ascontiguousarray((dq >= dk).astype(np.float32))


def kernel(hidden_state, attention_mask, W_attn, b_attn, W_proj, b_proj):
    global LAST_RESULT
    hs = np.asarray(hidden_state, np.float32)
    am = np.asarray(attention_mask, np.float32).reshape(B, S)
    wa = np.asarray(W_attn, np.float32)
    ba = np.asarray(b_attn, np.float32)
    wpr = np.asarray(W_proj, np.float32)
    bp = np.asarray(b_proj, np.float32)

    has_bv = bool(np.any(ba[2 * E:3 * E] != 0.0))
    key = ("k", has_bv)
    if key not in _CACHE:
        _CACHE[key] = _build(has_bv)
    nc = _CACHE[key]

    bf16 = ml_dtypes.bfloat16
    causal = _causal_tiles().astype(bf16)
    in_maps = []
    for core in range(NC):
        b = core // 2
        c0 = (core % 2) * EL
        in_maps.append({
            "xt": np.ascontiguousarray(hs[b].T).astype(bf16),
            "wq": np.ascontiguousarray(wa[:, c0:c0 + EL]).astype(bf16),
            "wk": np.ascontiguousarray(wa[:, E + c0:E + c0 + EL]).astype(bf16),
            "wv": np.ascontiguousarray(wa[:, 2 * E + c0:2 * E + c0 + EL]).astype(bf16),
            "wp": np.ascontiguousarray(wpr[c0:c0 + EL, :]).astype(bf16),
            "bq": np.ascontiguousarray(ba[c0:c0 + EL].reshape(4, P).T),
            "bk": np.ascontiguousarray(ba[E + c0:E + c0 + EL].reshape(4, P).T),
            "bv": np.ascontiguousarray(ba[2 * E + c0:2 * E + c0 + EL].reshape(4, P).T),
            "maskt": np.ascontiguousarray(am[b].reshape(NKT, P).T),
            "causal": causal,
            "ones": np.ones((P, 64), bf16),
        })

    from concourse.bass_utils import run_bass_kernel_spmd

    trace = os.environ.get("KERNEL_TRACE", "") == "1"
    res = run_bass_kernel_spmd(nc, in_maps, core_ids=list(range(NC)), trace=trace)
    LAST_RESULT = res

    full = np.empty((B, S, E), np.float32)
    for b in range(B):
        full[b] = outs_t = res.results[2 * b]["out"].T + res.results[2 * b + 1]["out"].T
        full[b] += bp
    return full


# revision 26
# speedup vs baseline: 1.0141x; 1.0141x over previous
"""Trainium2 Bass kernel for GPT2-style single attention layer.

Problem: B=4, S=2048, E=1024, H=16 heads, D=64.
  x = hidden @ W_attn + b_attn ; q,k,v = split(x)
  per head: softmax(causal(q k^T / 8) + mask) @ v
  out = merge @ W_proj + b_proj

Sharding over 8 cores: core i -> batch b = i//2, heads h0 = (i%2)*8 .. +8
(data parallel on B, tensor parallel over heads).  Each core's work is fully
local; the host sums the two partial projections per batch.

Dataflow is fully "transposed" so no on-chip transposes are ever needed:
  host feeds xT = hidden[b].T                       [E, S]
  Q^T,K^T = (Wq|Wk block)^T @ xT     -> [d, tok] per head   (W stationary)
  V       = xT_block^T @ Wv          -> [tok, d] natural    (xT stationary)
  S^T     = K^T_blk^T @ Q^T          -> [k, q]   (softmax dim on partitions)
  P^T     = exp(0.125*S^T + mask[k]) * causal01
  sums    = ones^T @ P^T             -> [1, q]  (ones-matmul, PSUM-accum)
  attn^T  = V_blk^T @ P^T            -> [d, q]  accumulated over k tiles
  norm    = attn^T * broadcast(1/sums)   (broadcast via K=1 ones-matmul)
  out^T   = Wp_blk^T @ attn^T        -> [col, tok]
Host transposes out^T back and sums core pairs + b_proj.

All matmuls run as float32r (full-rate fp32 path, 1 cycle/row at N>=256).
"""

import os
import ml_dtypes
import numpy as np

B, S, E, H, D = 4, 2048, 1024, 16, 64
NC = 8
HL = H // 2          # local heads per core
EL = HL * D          # local embedding slice = 512
P = 128              # partitions
QT = 512             # q tile width (f32 moving max)
NQT = S // QT        # 4 q tiles
NKT = S // P         # 16 k tiles
NET = E // P         # 8 e (contraction) tiles

_CACHE = {}
LAST_RESULT = None


def _build(has_bv: bool):
    from contextlib import ExitStack

    import concourse.tile as tile
    from concourse import bacc, mybir

    f32 = mybir.dt.float32
    f32r = mybir.dt.bfloat16  # matmul operand dtype (2-byte: full-rate moving operand)
    EXP = mybir.ActivationFunctionType.Exp

    nc = bacc.Bacc(
        "TRN2",
        target_bir_lowering=False,
        debug=False,
        enable_asserts=False,
        num_devices=NC,
    )

    def inp(name, shape, dt=f32):
        return nc.dram_tensor(name, shape, dt, kind="ExternalInput").ap()

    xt_d = inp("xt", [E, S], f32r)
    wq_d = inp("wq", [E, EL], f32r)
    wk_d = inp("wk", [E, EL], f32r)
    wv_d = inp("wv", [E, EL], f32r)
    wp_d = inp("wp", [EL, E], f32r)
    bq_d = inp("bq", [P, 4])
    bk_d = inp("bk", [P, 4])
    bv_d = inp("bv", [P, 4])
    maskt_d = inp("maskt", [P, NKT])
    causal_d = inp("causal", [P, P], f32r)
    ones_d = inp("ones", [P, 64], f32r)
    out_d = nc.dram_tensor("out", [E, S], f32, kind="ExternalOutput").ap()


    with tile.TileContext(nc) as tc, ExitStack() as ctx:
        const = ctx.enter_context(tc.tile_pool(name="const", bufs=1))
        big = ctx.enter_context(tc.tile_pool(name="big", bufs=1))
        wpool = ctx.enter_context(tc.tile_pool(name="wpool", bufs=1))
        xpool = ctx.enter_context(tc.tile_pool(name="xpool", bufs=1))
        ptpool = ctx.enter_context(tc.tile_pool(name="ptpool", bufs=1))
        aopool = ctx.enter_context(tc.tile_pool(name="aopool", bufs=1))
        ospool = ctx.enter_context(tc.tile_pool(name="ospool", bufs=1))
        rcpool = ctx.enter_context(tc.tile_pool(name="rcpool", bufs=1))
        aospool = ctx.enter_context(tc.tile_pool(name="aospool", bufs=1))
        psum = ctx.enter_context(tc.tile_pool(name="psum", bufs=1, space="PSUM"))


        # ---- persistent big buffers ----
        # Q^T / K^T: per head-pair p a [128, S] tile (partitions = 2 heads x 64 d)
        qt_tiles = [big.tile([P, S], f32r, name=f"qt{p}", tag=f"qt{p}") for p in range(4)]
        kt_tiles = [big.tile([P, S], f32r, name=f"kt{p}", tag=f"kt{p}") for p in range(4)]
        # V natural: 16 tiles [128 tok, 512 vcol]
        v_tiles = [big.tile([P, 8 * 65], f32r, name=f"v{t}", tag=f"v{t}") for t in range(NKT)]
        # W_proj: 4 partition tiles [128 elocal, 1024 col]
        wp_tiles = [big.tile([P, E], f32r, name=f"wp{p}", tag=f"wp{p}") for p in range(4)]

        # ---- weight tiles (shared slots) ----
        def load_w(dram, label):
            tiles = []
            for kt in range(NET):
                w = wpool.tile([P, EL], f32r, name=f"w_{label}{kt}", tag="w", bufs=24)
                nc.sync.dma_start(w[:], dram[kt * P:(kt + 1) * P, :])
                tiles.append(w)
            return tiles

        # DMA emission ordered by first use: wv + xT quarter 0 gate the first
        # compute; the rest stream in behind.
        x_tiles = [[None] * NQT for _ in range(NET)]

        def load_x_quarter(tq):
            for kt in range(NET):
                x = xpool.tile([P, QT], f32r, name=f"x{kt}_{tq}", tag=f"x{kt}_{tq}",
                               bufs=1)
                nc.sync.dma_start(x[:], xt_d[kt * P:(kt + 1) * P, tq * QT:(tq + 1) * QT])
                x_tiles[kt][tq] = x

        wv_t = load_w(wv_d, "v")
        load_x_quarter(0)
        wq_t = load_w(wq_d, "q")
        bq_t = const.tile([P, 4], f32, name="bq_t")
        nc.sync.dma_start(bq_t[:], bq_d[:])
        wk_t = load_w(wk_d, "k")
        bk_t = const.tile([P, 4], f32, name="bk_t")
        nc.sync.dma_start(bk_t[:], bk_d[:])
        maskt_t = const.tile([P, NKT], f32, name="maskt_t")
        nc.sync.dma_start(maskt_t[:], maskt_d[:])
        causal_t = const.tile([P, P], f32r, name="causal_t")
        nc.sync.dma_start(causal_t[:], causal_d[:])
        ones_t = const.tile([P, 64], f32r, name="ones_t")
        nc.sync.dma_start(ones_t[:], ones_d[:])
        bv_t = const.tile([P, 4], f32, name="bv_t")
        nc.sync.dma_start(bv_t[:], bv_d[:])
        for p in range(4):
            nc.sync.dma_start(wp_tiles[p][:], wp_d[p * P:(p + 1) * P, :])
        for tq in range(1, NQT):
            load_x_quarter(tq)

        def qk_pass_pair(tq, xq, ct):
            ps = psum.tile([P, QT], f32, name=f"psq{tq}_{ct}", tag="mm", bufs=2)
            for kt in range(NET):
                nc.tensor.matmul(ps[:], wq_t[kt][:, ct * P:(ct + 1) * P], xq[kt][:],
                                 start=(kt == 0), stop=(kt == NET - 1))
            nc.vector.tensor_scalar_add(
                qt_tiles[ct][:, tq * QT:(tq + 1) * QT], ps[:], bq_t[:, ct:ct + 1])
            ps2 = psum.tile([P, QT], f32, name=f"psk{tq}_{ct}", tag="mm", bufs=2)
            for kt in range(NET):
                nc.tensor.matmul(ps2[:], wk_t[kt][:, ct * P:(ct + 1) * P], xq[kt][:],
                                 start=(kt == 0), stop=(kt == NET - 1))
            nc.vector.tensor_scalar_add(
                kt_tiles[ct][:, tq * QT:(tq + 1) * QT], ps2[:], bk_t[:, ct:ct + 1])

        def v_pass(tq, xq):
            for tt in range(4):
                ps = psum.tile([P, EL], f32, name=f"psv{tq}_{tt}", tag="mm", bufs=2)
                for kt in range(NET):
                    nc.tensor.matmul(
                        ps[:],
                        xq[kt][:, tt * P:(tt + 1) * P],
                        wv_t[kt][:],
                        start=(kt == 0),
                        stop=(kt == NET - 1),
                    )
                vt = v_tiles[tq * 4 + tt]
                v8 = vt[:, 0:520].rearrange("p (a c) -> p a c", a=8, c=65)
                nc.vector.tensor_copy(
                    v8[:, :, 0:64], ps[:].rearrange("p (a c) -> p a c", a=8, c=64)
                )
                nc.sync.dma_start(
                    v8[:, :, 64:65],
                    ones_d[:, 0:8].rearrange("p (a o) -> p a o", a=8, o=1),
                )


        def attention(p, qt, sga, sgb):
            """Head pair p (heads 2p, 2p+1), q tile qt.

            Leaves attnout halves in an SBUF tile (f32) and the softmax
            denominators in rows 32*p of sga/sgb.  Normalization happens
            batched per qt in normalize()."""
            kt_max = 4 * (qt + 1)
            qsl = slice(qt * QT, (qt + 1) * QT)
            # row 64 of each av accumulates the softmax denominator (ones col)
            ava = psum.tile([65, QT], f32, name=f"ava{p}_{qt}", tag="ava", bufs=1)
            avb = psum.tile([65, QT], f32, name=f"avb{p}_{qt}", tag="avb", bufs=1)

            def av_sums(kt, pt, off):
                first, last = kt == 0, kt == kt_max - 1
                vva = v_tiles[kt][:, (2 * p) * 65:(2 * p + 1) * 65]
                vvb = v_tiles[kt][:, (2 * p + 1) * 65:(2 * p + 2) * 65]
                nc.tensor.matmul(ava[:, off:QT], vva, pt[:, off:QT],
                                 start=first, stop=last)
                nc.tensor.matmul(avb[:, off:QT], vvb, pt[:, QT + off:2 * QT],
                                 start=first, stop=last)

            pending = None
            for kt in range(kt_max):
                # diagonal tiles: only q columns >= off are unmasked
                diag = kt >= qt * 4
                off = (kt - qt * 4) * P if diag else 0
                kl = slice(kt * P, (kt + 1) * P)
                qv = slice(qt * QT + off, (qt + 1) * QT)
                st = psum.tile([P, 2 * QT], f32, name=f"st{p}_{qt}_{kt}",
                               tag="st", bufs=2)
                nc.tensor.matmul(st[:, off:QT], kt_tiles[p][0:64, kl],
                                 qt_tiles[p][0:64, qv])
                nc.tensor.matmul(st[:, QT + off:2 * QT], kt_tiles[p][64:128, kl],
                                 qt_tiles[p][64:128, qv])
                pt = ptpool.tile([P, 2 * QT], f32r, name=f"pt{p}_{qt}_{kt}",
                                 tag="pt", bufs=3)
                bias = maskt_t[:, kt:kt + 1]
                if not diag or off == 0:
                    nc.scalar.activation(pt[:], st[:], EXP, bias=bias, scale=0.125)
                else:
                    nc.scalar.activation(pt[:, off:QT], st[:, off:QT], EXP,
                                         bias=bias, scale=0.125)
                    nc.scalar.activation(pt[:, QT + off:2 * QT],
                                         st[:, QT + off:2 * QT], EXP,
                                         bias=bias, scale=0.125)
                if diag:
                    # triangular band at the leading 128 valid columns
                    nc.vector.tensor_mul(pt[:, off:off + P], pt[:, off:off + P],
                                         causal_t[:])
                    nc.vector.tensor_mul(pt[:, QT + off:QT + off + P],
                                         pt[:, QT + off:QT + off + P], causal_t[:])
                if pending is not None:
                    av_sums(*pending)
                pending = (kt, pt, off)
            av_sums(*pending)

            # drain PSUM immediately so the next pair's AV can start
            aos = aospool.tile([P, QT], f32, name=f"aos{p}_{qt}",
                               tag=f"aos{p}", bufs=2)
            nc.vector.tensor_copy(aos[0:64, :], ava[0:64, :])
            nc.vector.tensor_copy(aos[64:128, :], avb[0:64, :])
            row = 32 * p
            nc.vector.tensor_copy(sga[row:row + 1, :], ava[64:65, :])
            nc.vector.tensor_copy(sgb[row:row + 1, :], avb[64:65, :])
            return aos

        def normalize_pair(p, qt, sga, sgb, aos):
            """Immediate normalization of one pair (used for the last quarter)."""
            row = 32 * p
            rcf = rcpool.tile([97, QT], f32, name=f"rcfp{p}_{qt}", tag="rcf", bufs=1)
            rcg = rcpool.tile([97, QT], f32, name=f"rcgp{p}_{qt}", tag="rcg", bufs=1)
            nc.vector.reciprocal_approx_fast(rcf[:], sga[:])
            nc.vector.reciprocal_approx_fast(rcg[:], sgb[:])
            rca = rcpool.tile([97, QT], f32r, name=f"rcap{p}_{qt}", tag="rca", bufs=1)
            rcb = rcpool.tile([97, QT], f32r, name=f"rcbp{p}_{qt}", tag="rcb", bufs=1)
            nc.vector.tensor_copy(rca[:], rcf[:])
            nc.vector.tensor_copy(rcb[:], rcg[:])
            ao = aopool.tile([P, QT], f32r, name=f"aop{p}_{qt}", tag=f"ao{p}", bufs=2)
            for half, rcx in ((0, rca), (1, rcb)):
                rb = psum.tile([64, QT], f32, name=f"rbp{p}_{qt}_{half}",
                               tag="mm", bufs=2)
                nc.tensor.matmul(rb[:], ones_t[row:row + 1, 0:64],
                                 rcx[row:row + 1, :], tile_position=(row, 0))
                nc.vector.tensor_mul(ao[64 * half:64 * (half + 1), :], rb[:],
                                     aos[64 * half:64 * (half + 1), :])
            if has_bv:
                nc.vector.tensor_scalar_add(ao[:], ao[:], bv_t[:, p:p + 1])
            return ao

        def normalize(qt, sga, sgb, aos_tiles):
            """Batched softmax normalization for all 4 pairs of one q tile."""
            rcf = rcpool.tile([97, QT], f32, name=f"rcf{qt}", tag="rcf", bufs=1)
            rcg = rcpool.tile([97, QT], f32, name=f"rcg{qt}", tag="rcg", bufs=1)
            nc.vector.reciprocal_approx_fast(rcf[:], sga[:])
            nc.vector.reciprocal_approx_fast(rcg[:], sgb[:])
            rca = rcpool.tile([97, QT], f32r, name=f"rca{qt}", tag="rca", bufs=1)
            rcb = rcpool.tile([97, QT], f32r, name=f"rcb{qt}", tag="rcb", bufs=1)
            nc.vector.tensor_copy(rca[:], rcf[:])
            nc.vector.tensor_copy(rcb[:], rcg[:])
            ao_tiles = []
            for p in range(4):
                row = 32 * p
                ao = aopool.tile([P, QT], f32r, name=f"ao{p}_{qt}",
                                 tag=f"ao{p}", bufs=2)
                for half, rcx in ((0, rca), (1, rcb)):
                    rb = psum.tile([64, QT], f32, name=f"rb{p}_{qt}_{half}",
                                   tag="mm", bufs=2)
                    nc.tensor.matmul(rb[:], ones_t[row:row + 1, 0:64],
                                     rcx[row:row + 1, :], tile_position=(row, 0))
                    nc.vector.tensor_mul(ao[64 * half:64 * (half + 1), :], rb[:],
                                         aos_tiles[p][64 * half:64 * (half + 1), :])
                if has_bv:
                    nc.vector.tensor_scalar_add(ao[:], ao[:], bv_t[:, p:p + 1])
                ao_tiles.append(ao)
            return ao_tiles

        def proj(qt, ao_tiles):
            for ct in range(NET):
                ps = psum.tile([P, QT], f32, name=f"psp{qt}_{ct}", tag="mm", bufs=2)
                for p in range(4):
                    nc.tensor.matmul(
                        ps[:],
                        wp_tiles[p][:, ct * P:(ct + 1) * P],
                        ao_tiles[p][:],
                        start=(p == 0),
                        stop=(p == 3),
                    )
                osb = ospool.tile([P, QT], f32, name=f"os{qt}_{ct}", tag="os", bufs=2)
                nc.vector.tensor_copy(osb[:], ps[:])
                nc.sync.dma_start(out_d[ct * P:(ct + 1) * P, qt * QT:(qt + 1) * QT],
                                  osb[:])

        # ============ V/K passes interleaved with attention per quarter ============
        # normalize+proj of quarter tq are deferred into tq+1's attention so the
        # DVE/recip chain overlaps PE matmul work.
        pending_np = None
        for tq in range(NQT):
            xq = [x_tiles[kt][tq] for kt in range(NET)]
            v_pass(tq, xq)
            for ct in range(4):
                qk_pass_pair(tq, xq, ct)
            sga = rcpool.tile([97, QT], f32, name=f"sga{tq}", tag="sga", bufs=2)
            sgb = rcpool.tile([97, QT], f32, name=f"sgb{tq}", tag="sgb", bufs=2)
            aos_tiles = []
            for p in range(4):
                aos_tiles.append(attention(p, tq, sga, sgb))
                if p == 1 and pending_np is not None:
                    proj(pending_np[0], normalize(*pending_np))
                    pending_np = None
            pending_np = (tq, sga, sgb, aos_tiles)
        proj(pending_np[0], normalize(*pending_np))

    nc.compile()
    return nc


def _causal_tiles():
    """[128, 128] lower-triangular 0/1 band mask (dq >= dk)."""
    dk = np.arange(P)[:, None]
    dq = np.arange(P)[None, :]
    return np.ascontiguousarray((dq >= dk).astype(np.float32))


def kernel(hidden_state, attention_mask, W_attn, b_attn, W_proj, b_proj):
    global LAST_RESULT
    hs = np.asarray(hidden_state, np.float32)
    am = np.asarray(attention_mask, np.float32).reshape(B, S)
    wa = np.asarray(W_attn, np.float32)
    ba = np.asarray(b_attn, np.float32)
    wpr = np.asarray(W_proj, np.float32)
    bp = np.asarray(b_proj, np.float32)

    has_bv = bool(np.any(ba[2 * E:3 * E] != 0.0))
    key = ("k", has_bv)
    if key not in _CACHE:
        _CACHE[key] = _build(has_bv)
    nc = _CACHE[key]

    bf16 = ml_dtypes.bfloat16
    causal = _causal_tiles().astype(bf16)
    in_maps = []
    for core in range(NC):
        b = core // 2
        c0 = (core % 2) * EL
        in_maps.append({
            "xt": np.ascontiguousarray(hs[b].T).astype(bf16),
            "wq": np.ascontiguousarray(wa[:, c0:c0 + EL]).astype(bf16),
            "wk": np.ascontiguousarray(wa[:, E + c0:E + c0 + EL]).astype(bf16),
            "wv": np.ascontiguousarray(wa[:, 2 * E + c0:2 * E + c0 + EL]).astype(bf16),
            "wp": np.ascontiguousarray(wpr[c0:c0 + EL, :]).astype(bf16),
            "bq": np.ascontiguousarray(ba[c0:c0 + EL].reshape(4, P).T),
            "bk": np.ascontiguousarray(ba[E + c0:E + c0 + EL].reshape(4, P).T),
            "bv": np.ascontiguousarray(ba[2 * E + c0:2 * E + c0 + EL].reshape(4, P).T),
            "maskt": np.ascontiguousarray(am[b].reshape(NKT, P).T),
            "causal": causal,
            "ones": np.ones((P, 64), bf16),
        })

    from concourse.bass_utils import run_bass_kernel_spmd

    trace = os.environ.get("KERNEL_TRACE", "") == "1"
    res = run_bass_kernel_spmd(nc, in_maps, core_ids=list(range(NC)), trace=trace)
    LAST_RESULT = res

    full = np.empty((B, S, E), np.float32)
    for b in range(B):
        full[b] = outs_t = res.results[2 * b]["out"].T + res.results[2 * b + 1]["out"].T
        full[b] += bp
    return full


# revision 27
# speedup vs baseline: 1.0623x; 1.0475x over previous
"""Trainium2 Bass kernel for GPT2-style single attention layer.

Problem: B=4, S=2048, E=1024, H=16 heads, D=64.
  x = hidden @ W_attn + b_attn ; q,k,v = split(x)
  per head: softmax(causal(q k^T / 8) + mask) @ v
  out = merge @ W_proj + b_proj

Sharding over 8 cores: core i -> batch b = i//2, heads h0 = (i%2)*8 .. +8
(data parallel on B, tensor parallel over heads).  Each core's work is fully
local; the host sums the two partial projections per batch.

Dataflow is fully "transposed" so no on-chip transposes are ever needed:
  host feeds xT = hidden[b].T                       [E, S]
  Q^T,K^T = (Wq|Wk block)^T @ xT     -> [d, tok] per head   (W stationary)
  V       = xT_block^T @ Wv          -> [tok, d] natural    (xT stationary)
  S^T     = K^T_blk^T @ Q^T          -> [k, q]   (softmax dim on partitions)
  P^T     = exp(0.125*S^T + mask[k]) * causal01
  sums    = ones^T @ P^T             -> [1, q]  (ones-matmul, PSUM-accum)
  attn^T  = V_blk^T @ P^T            -> [d, q]  accumulated over k tiles
  norm    = attn^T * broadcast(1/sums)   (broadcast via K=1 ones-matmul)
  out^T   = Wp_blk^T @ attn^T        -> [col, tok]
Host transposes out^T back and sums core pairs + b_proj.

All matmuls run as float32r (full-rate fp32 path, 1 cycle/row at N>=256).
"""

import os
import ml_dtypes
import numpy as np

B, S, E, H, D = 4, 2048, 1024, 16, 64
NC = 8
HL = H // 2          # local heads per core
EL = HL * D          # local embedding slice = 512
P = 128              # partitions
QT = 512             # q tile width (f32 moving max)
NQT = S // QT        # 4 q tiles
NKT = S // P         # 16 k tiles
NET = E // P         # 8 e (contraction) tiles

_CACHE = {}
LAST_RESULT = None


def _build(has_bv: bool):
    from contextlib import ExitStack

    import concourse.tile as tile
    from concourse import bacc, mybir

    f32 = mybir.dt.float32
    f32r = mybir.dt.bfloat16  # matmul operand dtype (2-byte: full-rate moving operand)
    EXP = mybir.ActivationFunctionType.Exp

    nc = bacc.Bacc(
        "TRN2",
        target_bir_lowering=False,
        debug=False,
        enable_asserts=False,
        num_devices=NC,
    )

    def inp(name, shape, dt=f32):
        return nc.dram_tensor(name, shape, dt, kind="ExternalInput").ap()

    xt_d = inp("xt", [E, S], f32r)
    wq_d = inp("wq", [E, EL], f32r)
    wk_d = inp("wk", [E, EL], f32r)
    wv_d = inp("wv", [E, EL], f32r)
    wp_d = inp("wp", [EL, E], f32r)
    bq_d = inp("bq", [P, 4])
    bk_d = inp("bk", [P, 4])
    bv_d = inp("bv", [P, 4])
    maskt_d = inp("maskt", [P, NKT])
    causal_d = inp("causal", [P, P], f32r)
    ones_d = inp("ones", [P, 64], f32r)
    out_d = nc.dram_tensor("out", [E, S], f32, kind="ExternalOutput").ap()


    with tile.TileContext(nc) as tc, ExitStack() as ctx:
        const = ctx.enter_context(tc.tile_pool(name="const", bufs=1))
        big = ctx.enter_context(tc.tile_pool(name="big", bufs=1))
        wpool = ctx.enter_context(tc.tile_pool(name="wpool", bufs=1))
        xpool = ctx.enter_context(tc.tile_pool(name="xpool", bufs=1))
        ptpool = ctx.enter_context(tc.tile_pool(name="ptpool", bufs=1))
        aopool = ctx.enter_context(tc.tile_pool(name="aopool", bufs=1))
        ospool = ctx.enter_context(tc.tile_pool(name="ospool", bufs=1))
        rcpool = ctx.enter_context(tc.tile_pool(name="rcpool", bufs=1))
        aospool = ctx.enter_context(tc.tile_pool(name="aospool", bufs=1))
        psum = ctx.enter_context(tc.tile_pool(name="psum", bufs=1, space="PSUM"))


        # ---- persistent big buffers ----
        # Q^T / K^T: per head-pair p a [128, S] tile (partitions = 2 heads x 64 d)
        qt_tiles = [big.tile([P, S], f32r, name=f"qt{p}", tag=f"qt{p}") for p in range(4)]
        kt_tiles = [big.tile([P, S], f32r, name=f"kt{p}", tag=f"kt{p}") for p in range(4)]
        # V natural: 16 tiles [128 tok, 512 vcol]
        v_tiles = [big.tile([P, 8 * 65], f32r, name=f"v{t}", tag=f"v{t}") for t in range(NKT)]
        # W_proj: 4 partition tiles [128 elocal, 1024 col]
        wp_tiles = [big.tile([P, E], f32r, name=f"wp{p}", tag=f"wp{p}") for p in range(4)]

        # ---- weight tiles (shared slots) ----
        def load_w(dram, label):
            tiles = []
            for kt in range(NET):
                w = wpool.tile([P, EL], f32r, name=f"w_{label}{kt}", tag="w", bufs=24)
                nc.sync.dma_start(w[:], dram[kt * P:(kt + 1) * P, :])
                tiles.append(w)
            return tiles

        # DMA emission ordered by first use: wv + xT quarter 0 gate the first
        # compute; the rest stream in behind.
        x_tiles = [[None] * NQT for _ in range(NET)]

        def load_x_quarter(tq):
            for kt in range(NET):
                x = xpool.tile([P, QT], f32r, name=f"x{kt}_{tq}", tag=f"x{kt}_{tq}",
                               bufs=1)
                nc.sync.dma_start(x[:], xt_d[kt * P:(kt + 1) * P, tq * QT:(tq + 1) * QT])
                x_tiles[kt][tq] = x

        wv_t = load_w(wv_d, "v")
        load_x_quarter(0)
        wq_t = load_w(wq_d, "q")
        bq_t = const.tile([P, 4], f32, name="bq_t")
        nc.sync.dma_start(bq_t[:], bq_d[:])
        wk_t = load_w(wk_d, "k")
        bk_t = const.tile([P, 4], f32, name="bk_t")
        nc.sync.dma_start(bk_t[:], bk_d[:])
        maskt_t = const.tile([P, NKT], f32, name="maskt_t")
        nc.sync.dma_start(maskt_t[:], maskt_d[:])
        causal_t = const.tile([P, P], f32r, name="causal_t")
        nc.sync.dma_start(causal_t[:], causal_d[:])
        ones_t = const.tile([P, 64], f32r, name="ones_t")
        nc.sync.dma_start(ones_t[:], ones_d[:])
        bv_t = const.tile([P, 4], f32, name="bv_t")
        nc.sync.dma_start(bv_t[:], bv_d[:])
        for p in range(4):
            nc.sync.dma_start(wp_tiles[p][:], wp_d[p * P:(p + 1) * P, :])
        for tq in range(1, NQT):
            load_x_quarter(tq)


        # ---- per-group compute units (run directly or as PE fillers) ----
        done = set()

        def v_tt(tq, tt):
            key = ("v", tq, tt)
            if key in done:
                return
            done.add(key)
            ps = psum.tile([P, EL], f32, name=f"psv{tq}_{tt}", tag="mm", bufs=2)
            for kt in range(NET):
                nc.tensor.matmul(
                    ps[:], x_tiles[kt][tq][:, tt * P:(tt + 1) * P], wv_t[kt][:],
                    start=(kt == 0), stop=(kt == NET - 1))
            vt = v_tiles[tq * 4 + tt]
            v8 = vt[:, 0:520].rearrange("p (a c) -> p a c", a=8, c=65)
            nc.vector.tensor_copy(
                v8[:, :, 0:64], ps[:].rearrange("p (a c) -> p a c", a=8, c=64))
            nc.sync.dma_start(
                v8[:, :, 64:65],
                ones_d[:, 0:8].rearrange("p (a o) -> p a o", a=8, o=1))

        def q_ct(tq, ct):
            key = ("q", tq, ct)
            if key in done:
                return
            done.add(key)
            ps = psum.tile([P, QT], f32, name=f"psq{tq}_{ct}", tag="mm", bufs=2)
            for kt in range(NET):
                nc.tensor.matmul(ps[:], wq_t[kt][:, ct * P:(ct + 1) * P],
                                 x_tiles[kt][tq][:],
                                 start=(kt == 0), stop=(kt == NET - 1))
            nc.vector.tensor_scalar_add(
                qt_tiles[ct][:, tq * QT:(tq + 1) * QT], ps[:], bq_t[:, ct:ct + 1])

        def k_ct(tq, ct):
            key = ("k", tq, ct)
            if key in done:
                return
            done.add(key)
            ps = psum.tile([P, QT], f32, name=f"psk{tq}_{ct}", tag="mm", bufs=2)
            for kt in range(NET):
                nc.tensor.matmul(ps[:], wk_t[kt][:, ct * P:(ct + 1) * P],
                                 x_tiles[kt][tq][:],
                                 start=(kt == 0), stop=(kt == NET - 1))
            nc.vector.tensor_scalar_add(
                kt_tiles[ct][:, tq * QT:(tq + 1) * QT], ps[:], bk_t[:, ct:ct + 1])

        def proj_ct(qt, ct, ao_tiles):
            key = ("p", qt, ct)
            if key in done:
                return
            done.add(key)
            ps = psum.tile([P, QT], f32, name=f"psp{qt}_{ct}", tag="mm", bufs=2)
            for p in range(4):
                nc.tensor.matmul(ps[:], wp_tiles[p][:, ct * P:(ct + 1) * P],
                                 ao_tiles[p][:], start=(p == 0), stop=(p == 3))
            osb = ospool.tile([P, QT], f32, name=f"os{qt}_{ct}", tag="os", bufs=2)
            nc.vector.tensor_copy(osb[:], ps[:])
            nc.sync.dma_start(out_d[ct * P:(ct + 1) * P, qt * QT:(qt + 1) * QT],
                              osb[:])

        fillers = []

        def drain_filler():
            while fillers:
                fn = fillers.pop(0)
                if fn():  # returns True if it actually emitted work
                    return


        def attention(p, qt, sga, sgb):
            """Head pair p (heads 2p, 2p+1), q tile qt.

            Leaves attnout halves in an SBUF tile (f32) and the softmax
            denominators in rows 32*p of sga/sgb.  Normalization happens
            batched per qt in normalize()."""
            kt_max = 4 * (qt + 1)
            qsl = slice(qt * QT, (qt + 1) * QT)
            # row 64 of each av accumulates the softmax denominator (ones col)
            ava = psum.tile([65, QT], f32, name=f"ava{p}_{qt}", tag="ava", bufs=1)
            avb = psum.tile([65, QT], f32, name=f"avb{p}_{qt}", tag="avb", bufs=1)

            def av_sums(kt, pt, off):
                first, last = kt == 0, kt == kt_max - 1
                vva = v_tiles[kt][:, (2 * p) * 65:(2 * p + 1) * 65]
                vvb = v_tiles[kt][:, (2 * p + 1) * 65:(2 * p + 2) * 65]
                nc.tensor.matmul(ava[:, off:QT], vva, pt[:, off:QT],
                                 start=first, stop=last)
                nc.tensor.matmul(avb[:, off:QT], vvb, pt[:, QT + off:2 * QT],
                                 start=first, stop=last)

            pending = None
            for kt in range(kt_max):
                # diagonal tiles: only q columns >= off are unmasked
                diag = kt >= qt * 4
                off = (kt - qt * 4) * P if diag else 0
                kl = slice(kt * P, (kt + 1) * P)
                qv = slice(qt * QT + off, (qt + 1) * QT)
                st = psum.tile([P, 2 * QT], f32, name=f"st{p}_{qt}_{kt}",
                               tag="st", bufs=2)
                nc.tensor.matmul(st[:, off:QT], kt_tiles[p][0:64, kl],
                                 qt_tiles[p][0:64, qv])
                nc.tensor.matmul(st[:, QT + off:2 * QT], kt_tiles[p][64:128, kl],
                                 qt_tiles[p][64:128, qv])
                pt = ptpool.tile([P, 2 * QT], f32r, name=f"pt{p}_{qt}_{kt}",
                                 tag="pt", bufs=3)
                bias = maskt_t[:, kt:kt + 1]
                if not diag or off == 0:
                    nc.scalar.activation(pt[:], st[:], EXP, bias=bias, scale=0.125)
                else:
                    nc.scalar.activation(pt[:, off:QT], st[:, off:QT], EXP,
                                         bias=bias, scale=0.125)
                    nc.scalar.activation(pt[:, QT + off:2 * QT],
                                         st[:, QT + off:2 * QT], EXP,
                                         bias=bias, scale=0.125)
                if diag:
                    # triangular band at the leading 128 valid columns
                    nc.vector.tensor_mul(pt[:, off:off + P], pt[:, off:off + P],
                                         causal_t[:])
                    nc.vector.tensor_mul(pt[:, QT + off:QT + off + P],
                                         pt[:, QT + off:QT + off + P], causal_t[:])
                if pending is not None:
                    av_sums(*pending)
                    if kt % 2 == 0:
                        drain_filler()
                pending = (kt, pt, off)
            av_sums(*pending)

            # drain PSUM immediately so the next pair's AV can start
            aos = aospool.tile([P, QT], f32, name=f"aos{p}_{qt}",
                               tag=f"aos{p}", bufs=2)
            nc.vector.tensor_copy(aos[0:64, :], ava[0:64, :])
            nc.vector.tensor_copy(aos[64:128, :], avb[0:64, :])
            row = 32 * p
            nc.vector.tensor_copy(sga[row:row + 1, :], ava[64:65, :])
            nc.vector.tensor_copy(sgb[row:row + 1, :], avb[64:65, :])
            return aos

        def normalize_pair(p, qt, sga, sgb, aos):
            """Immediate normalization of one pair (used for the last quarter)."""
            row = 32 * p
            rcf = rcpool.tile([97, QT], f32, name=f"rcfp{p}_{qt}", tag="rcf", bufs=1)
            rcg = rcpool.tile([97, QT], f32, name=f"rcgp{p}_{qt}", tag="rcg", bufs=1)
            nc.vector.reciprocal_approx_fast(rcf[:], sga[:])
            nc.vector.reciprocal_approx_fast(rcg[:], sgb[:])
            rca = rcpool.tile([97, QT], f32r, name=f"rcap{p}_{qt}", tag="rca", bufs=1)
            rcb = rcpool.tile([97, QT], f32r, name=f"rcbp{p}_{qt}", tag="rcb", bufs=1)
            nc.vector.tensor_copy(rca[:], rcf[:])
            nc.vector.tensor_copy(rcb[:], rcg[:])
            ao = aopool.tile([P, QT], f32r, name=f"aop{p}_{qt}", tag=f"ao{p}", bufs=2)
            for half, rcx in ((0, rca), (1, rcb)):
                rb = psum.tile([64, QT], f32, name=f"rbp{p}_{qt}_{half}",
                               tag="mm", bufs=2)
                nc.tensor.matmul(rb[:], ones_t[row:row + 1, 0:64],
                                 rcx[row:row + 1, :], tile_position=(row, 0))
                nc.vector.tensor_mul(ao[64 * half:64 * (half + 1), :], rb[:],
                                     aos[64 * half:64 * (half + 1), :])
            if has_bv:
                nc.vector.tensor_scalar_add(ao[:], ao[:], bv_t[:, p:p + 1])
            return ao

        def normalize(qt, sga, sgb, aos_tiles):
            """Batched softmax normalization for all 4 pairs of one q tile."""
            rcf = rcpool.tile([97, QT], f32, name=f"rcf{qt}", tag="rcf", bufs=1)
            rcg = rcpool.tile([97, QT], f32, name=f"rcg{qt}", tag="rcg", bufs=1)
            nc.vector.reciprocal_approx_fast(rcf[:], sga[:])
            nc.vector.reciprocal_approx_fast(rcg[:], sgb[:])
            rca = rcpool.tile([97, QT], f32r, name=f"rca{qt}", tag="rca", bufs=1)
            rcb = rcpool.tile([97, QT], f32r, name=f"rcb{qt}", tag="rcb", bufs=1)
            nc.vector.tensor_copy(rca[:], rcf[:])
            nc.vector.tensor_copy(rcb[:], rcg[:])
            ao_tiles = []
            for p in range(4):
                row = 32 * p
                ao = aopool.tile([P, QT], f32r, name=f"ao{p}_{qt}",
                                 tag=f"ao{p}", bufs=2)
                for half, rcx in ((0, rca), (1, rcb)):
                    rb = psum.tile([64, QT], f32, name=f"rb{p}_{qt}_{half}",
                                   tag="mm", bufs=2)
                    nc.tensor.matmul(rb[:], ones_t[row:row + 1, 0:64],
                                     rcx[row:row + 1, :], tile_position=(row, 0))
                    nc.vector.tensor_mul(ao[64 * half:64 * (half + 1), :], rb[:],
                                         aos_tiles[p][64 * half:64 * (half + 1), :])
                if has_bv:
                    nc.vector.tensor_scalar_add(ao[:], ao[:], bv_t[:, p:p + 1])
                ao_tiles.append(ao)
            return ao_tiles


        # ============ filler-queue main schedule ============
        # Attention k-loops are ACT(exp)-paced; PE idle slots are filled with
        # independent matmul groups: next quarter's V/Q/K and deferred proj.
        def mkfiller(fn, *args):
            def run():
                before = len(done)
                fn(*args)
                return len(done) != before
            return run

        pending_np = None
        for tq in range(NQT):
            # mandatory prelude for this quarter (no-ops if filler-drained)
            for tt in range(4):
                v_tt(tq, tt)
            for ct in range(4):
                q_ct(tq, ct)
                k_ct(tq, ct)
            # queue next quarter's V/Q/K as fillers
            if tq + 1 < NQT:
                for tt in range(4):
                    fillers.append(mkfiller(v_tt, tq + 1, tt))
                for ct in range(4):
                    fillers.append(mkfiller(q_ct, tq + 1, ct))
                    fillers.append(mkfiller(k_ct, tq + 1, ct))
            sga = rcpool.tile([97, QT], f32, name=f"sga{tq}", tag="sga", bufs=2)
            sgb = rcpool.tile([97, QT], f32, name=f"sgb{tq}", tag="sgb", bufs=2)
            aos_tiles = []
            for p in range(4):
                aos_tiles.append(attention(p, tq, sga, sgb))
                if p == 1 and pending_np is not None:
                    qt_prev, ao_prev = pending_np[0], normalize(*pending_np)
                    for ct in range(NET):
                        fillers.append(mkfiller(proj_ct, qt_prev, ct, ao_prev))
                    pending_np = None
            pending_np = (tq, sga, sgb, aos_tiles)
        # final: leftover fillers, then last quarter's normalize + proj
        while fillers:
            fillers.pop(0)()
        qt_last, ao_last = pending_np[0], normalize(*pending_np)
        for ct in range(NET):
            proj_ct(qt_last, ct, ao_last)

    nc.compile()
    return nc


def _causal_tiles():
    """[128, 128] lower-triangular 0/1 band mask (dq >= dk)."""
    dk = np.arange(P)[:, None]
    dq = np.arange(P)[None, :]
    return np.ascontiguousarray((dq >= dk).astype(np.float32))


def kernel(hidden_state, attention_mask, W_attn, b_attn, W_proj, b_proj):
    global LAST_RESULT
    hs = np.asarray(hidden_state, np.float32)
    am = np.asarray(attention_mask, np.float32).reshape(B, S)
    wa = np.asarray(W_attn, np.float32)
    ba = np.asarray(b_attn, np.float32)
    wpr = np.asarray(W_proj, np.float32)
    bp = np.asarray(b_proj, np.float32)

    has_bv = bool(np.any(ba[2 * E:3 * E] != 0.0))
    key = ("k", has_bv)
    if key not in _CACHE:
        _CACHE[key] = _build(has_bv)
    nc = _CACHE[key]

    bf16 = ml_dtypes.bfloat16
    causal = _causal_tiles().astype(bf16)
    in_maps = []
    for core in range(NC):
        b = core // 2
        c0 = (core % 2) * EL
        in_maps.append({
            "xt": np.ascontiguousarray(hs[b].T).astype(bf16),
            "wq": np.ascontiguousarray(wa[:, c0:c0 + EL]).astype(bf16),
            "wk": np.ascontiguousarray(wa[:, E + c0:E + c0 + EL]).astype(bf16),
            "wv": np.ascontiguousarray(wa[:, 2 * E + c0:2 * E + c0 + EL]).astype(bf16),
            "wp": np.ascontiguousarray(wpr[c0:c0 + EL, :]).astype(bf16),
            "bq": np.ascontiguousarray(ba[c0:c0 + EL].reshape(4, P).T),
            "bk": np.ascontiguousarray(ba[E + c0:E + c0 + EL].reshape(4, P).T),
            "bv": np.ascontiguousarray(ba[2 * E + c0:2 * E + c0 + EL].reshape(4, P).T),
            "maskt": np.ascontiguousarray(am[b].reshape(NKT, P).T),
            "causal": causal,
            "ones": np.ones((P, 64), bf16),
        })

    from concourse.bass_utils import run_bass_kernel_spmd

    trace = os.environ.get("KERNEL_TRACE", "") == "1"
    res = run_bass_kernel_spmd(nc, in_maps, core_ids=list(range(NC)), trace=trace)
    LAST_RESULT = res

    full = np.empty((B, S, E), np.float32)
    for b in range(B):
        full[b] = outs_t = res.results[2 * b]["out"].T + res.results[2 * b + 1]["out"].T
        full[b] += bp
    return full
